# revision 33
# baseline (speedup 1.0000x reference)
"""Trainium2 Bass kernel for nn_CustomLoss_Z_B_25031069401264.

Computes the 6-scalar custom loss (divergence fluxes + variances, 5x5x5
median-filter smoothness losses) for inputs pred_b [1,3,96,96,96],
pred_z [1,1,96,96,96], targets [1,3,96,96,96].

Strategy:
  - D axis sharded across 8 cores (12 output planes each). Host pre-slices
    overlapping input slabs with all D-reflects resolved, so the SPMD
    program is identical on every core.
  - On-chip layout: W on the partition axis, (D-plane, H) in the free dim.
    W-shifts are materialized as 5 DMA-aligned copies; H and D shifts are
    free-dim AP offsets.
  - Exact medians via shared sorting networks: sort5 along W -> sorted-25
    per voxel (Batcher merges along H) -> paired D-phase: merge(25,25) at
    odd positions, pruned merge(50,50) to ranks 37..62 shared by two
    consecutive output planes, then a 26-term min/max rank select.
  - Each core emits per-partition partial sums [128,16]; host combines in
    float64 and returns the 6 scalars.
"""

import numpy as np
from concourse import bass, mybir
from concourse.tile import TileContext
from concourse.bass_utils import run_bass_kernel_spmd

F32 = mybir.dt.float32
F16 = mybir.dt.float16
Alu = mybir.AluOpType
ActF = mybir.ActivationFunctionType

NCORES = 8
DS = 12          # output D planes per core
DIN = 16         # median field slab planes per core: [12c-2, 12c+14)
HC = 48          # H chunk size for the median phase-2 (sorted-25)
HC3 = 96         # H chunk size for the median phase-3 (D-selection)
HCF = 32         # H chunk size for the flux pass
GP_MOD = 0       # every GP_MOD-th median CE op goes to GpSimd (0 = disable)

# ---------------------------------------------------------------------------
# sorting / merge networks
# ---------------------------------------------------------------------------
SORT5 = [(0, 1), (3, 4), (2, 4), (2, 3), (1, 4), (0, 3), (0, 2), (1, 3), (1, 2)]


def _oem_merge(A, B, ces):
    if not A:
        return list(B)
    if not B:
        return list(A)
    if len(A) == 1 and len(B) == 1:
        ces.append((A[0], B[0]))
        return [A[0], B[0]]
    E = _oem_merge(A[0::2], B[0::2], ces)
    O = _oem_merge(A[1::2], B[1::2], ces)
    out = [E[0]]
    i = 0
    while i < len(O) and i + 1 < len(E):
        ces.append((O[i], E[i + 1]))
        out.append(O[i])
        out.append(E[i + 1])
        i += 1
    out.extend(O[i:])
    out.extend(E[i + 1:])
    return out


def _prune(ces, needed_wires):
    needed = set(needed_wires)
    kept = []
    for (u, v) in reversed(ces):
        nm, nM = u in needed, v in needed
        if not (nm or nM):
            continue
        kept.append((u, v, nm, nM))
        needed.add(u)
        needed.add(v)
    kept.reverse()
    return kept


_MERGE_CACHE = {}


def merge_net(m, n, needed_ranks=None):
    """Returns (pruned_ces, order). Wires 0..m-1 = sorted A, m..m+n-1 = sorted B."""
    key = (m, n, tuple(needed_ranks) if needed_ranks is not None else None)
    if key in _MERGE_CACHE:
        return _MERGE_CACHE[key]
    ces = []
    order = _oem_merge(list(range(m)), list(range(m, m + n)), ces)
    if needed_ranks is None:
        pruned = _prune(ces, set(range(m + n)))
    else:
        pruned = _prune(ces, {order[r] for r in needed_ranks})
    _MERGE_CACHE[key] = (pruned, order)
    return pruned, order


def peak_live(ces, n_wires, final_needed=None):
    version = {i: ('ext', i) for i in range(n_wires)}
    reads = {}
    tid = 0
    creations = []
    for step, (u, v, nm, nM) in enumerate(ces):
        for w in (u, v):
            t = version[w]
            if t[0] == 'tile':
                reads[t[1]] = step
        if nm:
            creations.append((step, tid))
            version[u] = ('tile', tid)
            tid += 1
        if nM:
            creations.append((step, tid))
            version[v] = ('tile', tid)
            tid += 1
    end = len(ces)
    finals = set()
    fw = final_needed if final_needed is not None else range(n_wires)
    for wdx in fw:
        t = version[wdx]
        if t[0] == 'tile':
            finals.add(t[1])
    events = []
    for (cs, t) in creations:
        last = end if t in finals else reads.get(t, cs)
        events.append((cs, 1))
        events.append((last + 0.5, -1))
    events.sort()
    cur = peak = 0
    for (_, d) in events:
        cur += d
        peak = max(peak, cur)
    return peak


# ---------------------------------------------------------------------------
# emitter helpers
# ---------------------------------------------------------------------------

def emit_ces(nc, pool, tag, bufs, shape, wires, ces):
    """SSA compare-exchange emission. wires: dict idx -> AP (pre-sliced to a
    common extent == shape[1:]). Updates wires in place."""
    for (u, v, nm, nM) in ces:
        a, b = wires[u], wires[v]
        if nm:
            tmn = pool.tile(shape, F32, tag=tag, bufs=bufs, name=f"{tag}_mn")
            nc.vector.tensor_tensor(out=tmn[:], in0=a, in1=b, op=Alu.min)
        if nM:
            tmx = pool.tile(shape, F32, tag=tag, bufs=bufs, name=f"{tag}_mx")
            nc.vector.tensor_tensor(out=tmx[:], in0=a, in1=b, op=Alu.max)
        if nm:
            wires[u] = tmn[:]
        if nM:
            wires[v] = tmx[:]


def refl(d, n):
    if d < 0:
        return -d
    if d >= n:
        return 2 * (n - 1) - d
    return d


FILTER_ORDER = ['bxp', 'byp', 'bxm', 'bym', 'jx', 'jy', 'jz']
SLOT = {'f_p': 0, 'f2_p': 1, 'f_t': 2, 'f2_t': 3}
FIELD_HEXT = {'bxp': 96, 'byp': 96, 'bxm': 96, 'bym': 96,
              'jx': 95, 'jy': 96, 'jz': 95}


def chunks_for(Hext, hc):
    out = []
    h0 = 0
    while h0 < Hext:
        out.append((h0, min(hc, Hext - h0)))
        h0 += hc
    return out


def _build_med_slots():
    """Column map for the per-(field, chunk, par, block) ACT accumulators.
    block 'l' (last D plane of par 1) is split out for jy/jz so the host can
    drop core 7's out-of-range plane."""
    slots = {}
    col = 4
    for fname in FILTER_ORDER:
        for (h0, _) in chunks_for(FIELD_HEXT[fname], HC3):
            for par in (0, 1):
                blks = ('m', 'l') if (par == 1 and fname in ('jy', 'jz')) \
                    else ('m',)
                for blk in blks:
                    slots[(fname, h0, par, blk)] = col
                    col += 1
    return slots, col


MED_SLOT, _NCOL = _build_med_slots()


def _build_flux_slots(col):
    slots = {}
    for variant in ('p', 't'):
        for (h0, _) in chunks_for(95, HCF):
            for comp in ('f', 'f2'):
                for blk in ('m', 'l'):
                    slots[(variant, h0, comp, blk)] = col
                    col += 1
    return slots, col


FLUX_SLOT, _NCOL2 = _build_flux_slots(_NCOL)
NSLOT = 64
assert _NCOL2 <= NSLOT


# ---------------------------------------------------------------------------
# program builder (SPMD; identical for all cores)
# ---------------------------------------------------------------------------

def build_program():
    nc = bass.Bass()

    A = {f: nc.declare_dram_parameter(f"A_{f}", [96, DIN, 96], F32, isOutput=False)
         for f in ['bxt', 'byt', 'bxp', 'byp', 'bzp']}
    J0 = {f: nc.declare_dram_parameter(f"J0_{f}", [96, DIN, 96], F32, isOutput=False)
          for f in ['bxt', 'byt', 'bxp', 'byp', 'bzp']}
    J1 = {f: nc.declare_dram_parameter(f"J1_{f}", [96, DIN, 96], F32, isOutput=False)
          for f in ['bxt', 'byt', 'bxp', 'byp', 'bzp']}
    FX = {f: nc.declare_dram_parameter(f"Fx_{f}", [96, 13, 96], F32, isOutput=False)
          for f in ['bxp', 'byp', 'bxt', 'byt', 'bzt', 'z']}
    AH = {f: nc.declare_dram_parameter(f"Ah_{f}", [96, DIN, 96], F16, isOutput=False)
          for f in ['bxp', 'byp']}
    maskp_ext = nc.declare_dram_parameter("maskp", [128, 1], F32, isOutput=False)
    out_ext = nc.declare_dram_parameter("out", [128, NSLOT], F32, isOutput=True)

    scr = {
        'bxm': nc.dram_tensor("scr_bxm", [96, DIN, 96], F16),
        'bym': nc.dram_tensor("scr_bym", [96, DIN, 96], F16),
        'jx': nc.dram_tensor("scr_jx", [95, DIN, 95], F16),
        'jy': nc.dram_tensor("scr_jy", [95, DIN, 96], F16),
        'jz': nc.dram_tensor("scr_jz", [96, DIN, 95], F16),
    }

    with TileContext(nc) as tc:
        with tc.tile_pool(name="acc", bufs=1) as accpool:
            acc = accpool.tile([128, NSLOT], F32, name="acc")
            nc.vector.memset(acc[:], 0.0)
            maskp = accpool.tile([128, 1], F32, name="maskp_t")
            nc.sync.dma_start(out=maskp[:], in_=maskp_ext[:])

            _emit_pass1_fields(nc, tc, A, J0, J1, scr)
            _emit_pass1_flux(nc, tc, FX, acc, maskp)
            _emit_pass2_medians(nc, tc, AH, scr, acc)

            nc.sync.dma_start(out=out_ext[:], in_=acc[:])
    return nc


def _legalize_multiwaits(nc):
    """This walrus build only supports ONE sync-wait per instruction. Move
    excess waits onto injected same-engine NoOps (sequencer stalls there,
    preserving ordering exactly)."""
    ctr = 0
    for fn in nc.m.functions:
        for bb in fn.blocks:
            insts = bb.instructions
            new = []
            changed = False
            for inst in insts:
                si = inst.sync_info
                if si is not None and si.on_wait and len(si.on_wait) > 1:
                    waits = list(si.on_wait)
                    for w in waits[:-1]:
                        nop = mybir.InstNoOp(name=f"waitnop_{ctr}")
                        ctr += 1
                        nop.engine = inst.engine
                        nop.sync_info = mybir.SyncInfo(on_wait=[w], on_update=[])
                        new.append(nop)
                    inst.sync_info = mybir.SyncInfo(on_wait=[waits[-1]],
                                                    on_update=list(si.on_update))
                    changed = True
                new.append(inst)
            if changed:
                bb.instructions = new
    return nc


def _make_mask(nc, tc, pool_persist, persist_tag, shape, nr, bxp, byp, bxt, byt,
               dt=F32):
    """mask = 2*(bxp*bxt + byp*byt > 0) - 1 into a persistent tile, using a
    transient inner pool. All ops on partition rows [0:nr)."""
    mk = pool_persist.tile(shape, dt, tag=persist_tag, bufs=2, name=persist_tag)
    with nc.tc.tile_pool(name=f"mk_{persist_tag}", bufs=1) as mp:
        t1 = mp.tile(shape, dt, tag="mt", bufs=5, name="mt_1")
        nc.vector.tensor_tensor(out=t1[0:nr], in0=bxp, in1=bxt, op=Alu.mult)
        t2 = mp.tile(shape, dt, tag="mt", bufs=5, name="mt_2")
        nc.vector.tensor_tensor(out=t2[0:nr], in0=byp, in1=byt, op=Alu.mult)
        t3 = mp.tile(shape, dt, tag="mt", bufs=5, name="mt_3")
        nc.vector.tensor_tensor(out=t3[0:nr], in0=t1[0:nr], in1=t2[0:nr], op=Alu.add)
        m = mp.tile(shape, dt, tag="mt", bufs=5, name="mt_4")
        nc.vector.tensor_scalar(out=m[0:nr], in0=t3[0:nr], scalar1=0.0, scalar2=None,
                                op0=Alu.is_gt)
        nc.vector.tensor_scalar(out=mk[0:nr], in0=m[0:nr], scalar1=2.0, scalar2=-1.0,
                                op0=Alu.mult, op1=Alu.add)
    return mk


def _emit_pass1_fields(nc, tc, A, J0, J1, scr):
    """Compute bxm, bym (A-arranged) and jx, jy, jz; write DRAM scratch."""
    nc.tc = tc
    shape = [128, DIN, 96]
    sh95 = [128, DIN, 95]

    with tc.tile_pool(name="p1a", bufs=1) as pool:
        ta = {}
        for f in ['bxt', 'byt', 'bxp', 'byp', 'bzp']:
            t = pool.tile(shape, F32, tag=f"A_{f}", bufs=1, name=f"tA_{f}")
            nc.sync.dma_start(out=t[0:96], in_=A[f][:])
            ta[f] = t
        maskA = _make_mask(nc, tc, pool, "maskA", shape, 96, ta['bxp'][0:96],
                           ta['byp'][0:96], ta['bxt'][0:96], ta['byt'][0:96])
        bxmA = pool.tile(shape, F32, tag="bxmA", bufs=1, name="bxmA")
        nc.vector.tensor_tensor(out=bxmA[0:96], in0=ta['bxt'][0:96],
                                in1=maskA[0:96], op=Alu.mult)
        bymA = pool.tile(shape, F32, tag="bymA", bufs=1, name="bymA")
        nc.vector.tensor_tensor(out=bymA[0:96], in0=ta['byt'][0:96],
                                in1=maskA[0:96], op=Alu.mult)
        bxmA16 = pool.tile(shape, F16, tag="bxmA16", bufs=1, name="bxmA16")
        nc.scalar.copy(out=bxmA16[0:96], in_=bxmA[0:96])
        bymA16 = pool.tile(shape, F16, tag="bymA16", bufs=1, name="bymA16")
        nc.scalar.copy(out=bymA16[0:96], in_=bymA[0:96])
        nc.sync.dma_start(out=scr['bxm'][:], in_=bxmA16[0:96])
        nc.sync.dma_start(out=scr['bym'][:], in_=bymA16[0:96])

        # jx = 0.5*[(dyBz + dyBz_s) - (dzBy_h + dzBy_h1)], valid rows 0..94
        with tc.tile_pool(name="p1a_jx", bufs=1) as jp:
            bzpS = jp.tile(shape, F32, tag="tmp", bufs=4, name="bzpS")
            nc.sync.dma_start(out=bzpS[0:95], in_=ta['bzp'][1:96])
            bymS = jp.tile(shape, F32, tag="tmp", bufs=4, name="bymS")
            nc.sync.dma_start(out=bymS[0:95], in_=bymA[1:96])

            def t95(name):
                return jp.tile(sh95, F32, tag="t95", bufs=5, name=name)

            dy0 = t95("dy0")
            nc.vector.tensor_tensor(out=dy0[0:95], in0=ta['bzp'][0:95, :, 0:95],
                                    in1=ta['bzp'][0:95, :, 1:96], op=Alu.subtract)
            dy1 = t95("dy1")
            nc.vector.tensor_tensor(out=dy1[0:95], in0=bzpS[0:95, :, 0:95],
                                    in1=bzpS[0:95, :, 1:96], op=Alu.subtract)
            u = t95("u")
            nc.vector.tensor_tensor(out=u[0:95], in0=dy0[0:95], in1=dy1[0:95],
                                    op=Alu.add)
            dzby = jp.tile(shape, F32, tag="tmp", bufs=4, name="dzby")
            nc.vector.tensor_tensor(out=dzby[0:95], in0=bymA[0:95], in1=bymS[0:95],
                                    op=Alu.subtract)
            v = t95("v")
            nc.vector.tensor_tensor(out=v[0:95], in0=dzby[0:95, :, 0:95],
                                    in1=dzby[0:95, :, 1:96], op=Alu.add)
            t = t95("t")
            nc.vector.tensor_tensor(out=t[0:95], in0=u[0:95], in1=v[0:95],
                                    op=Alu.subtract)
            jx = jp.tile(sh95, F16, tag="jx16", bufs=1, name="jx")
            nc.vector.tensor_scalar(out=jx[0:95], in0=t[0:95], scalar1=0.5,
                                    scalar2=None, op0=Alu.mult)
            nc.sync.dma_start(out=scr['jx'][:], in_=jx[0:95])

    with tc.tile_pool(name="p1b", bufs=1) as pool:
        keep = {}
        for (pref, J) in [("0", J0), ("1", J1)]:
            with tc.tile_pool(name=f"p1b_in{pref}", bufs=1) as ip:
                tj = {}
                for f in ['bxt', 'byt', 'bxp', 'byp']:
                    t = ip.tile(shape, F32, tag=f"J_{f}", bufs=1, name=f"tJ{pref}_{f}")
                    nc.sync.dma_start(out=t[0:96], in_=J[f][:])
                    tj[f] = t
                mk = _make_mask(nc, tc, ip, f"maskJ{pref}", shape, 96,
                                tj['bxp'][0:96], tj['byp'][0:96],
                                tj['bxt'][0:96], tj['byt'][0:96])
                bxm = pool.tile(shape, F32, tag=f"bxm{pref}", bufs=1,
                                name=f"bxm{pref}")
                nc.vector.tensor_tensor(out=bxm[0:96], in0=tj['bxt'][0:96],
                                        in1=mk[0:96], op=Alu.mult)
                bym = pool.tile(shape, F32, tag=f"bym{pref}", bufs=1,
                                name=f"bym{pref}")
                nc.vector.tensor_tensor(out=bym[0:96], in0=tj['byt'][0:96],
                                        in1=mk[0:96], op=Alu.mult)
                keep[f"bxm{pref}"] = bxm
                keep[f"bym{pref}"] = bym

        bzp0 = pool.tile(shape, F32, tag="bzp0", bufs=1, name="bzp0")
        nc.sync.dma_start(out=bzp0[0:96], in_=J0['bzp'][:])
        bzp1 = pool.tile(shape, F32, tag="bzp1", bufs=1, name="bzp1")
        nc.sync.dma_start(out=bzp1[0:96], in_=J1['bzp'][:])

        with tc.tile_pool(name="p1b_j", bufs=1) as jp:
            def tmp(name):
                return jp.tile(shape, F32, tag="tmp", bufs=7, name=name)

            def t95(name):
                return jp.tile(sh95, F32, tag="t95", bufs=6, name=name)

            bxm0, bxm1 = keep["bxm0"], keep["bxm1"]
            bym0, bym1 = keep["bym0"], keep["bym1"]
            # jy = 0.5*[(dzBx0 + dzBx1) - (dxz + dxz_s)], valid rows 0..94
            bxm0S = tmp("bxm0S")
            nc.sync.dma_start(out=bxm0S[0:95], in_=bxm0[1:96])
            bxm1S = tmp("bxm1S")
            nc.sync.dma_start(out=bxm1S[0:95], in_=bxm1[1:96])
            dzbx0 = tmp("dzbx0")
            nc.vector.tensor_tensor(out=dzbx0[0:95], in0=bxm0[0:95], in1=bxm0S[0:95],
                                    op=Alu.subtract)
            dzbx1 = tmp("dzbx1")
            nc.vector.tensor_tensor(out=dzbx1[0:95], in0=bxm1[0:95], in1=bxm1S[0:95],
                                    op=Alu.subtract)
            a = tmp("a")
            nc.vector.tensor_tensor(out=a[0:95], in0=dzbx0[0:95], in1=dzbx1[0:95],
                                    op=Alu.add)
            dxz = tmp("dxz")
            nc.vector.tensor_tensor(out=dxz[0:96], in0=bzp0[0:96], in1=bzp1[0:96],
                                    op=Alu.subtract)
            dxzS = tmp("dxzS")
            nc.sync.dma_start(out=dxzS[0:95], in_=dxz[1:96])
            b = tmp("b")
            nc.vector.tensor_tensor(out=b[0:95], in0=dxz[0:95], in1=dxzS[0:95],
                                    op=Alu.add)
            t2 = tmp("t2")
            nc.vector.tensor_tensor(out=t2[0:95], in0=a[0:95], in1=b[0:95],
                                    op=Alu.subtract)
            jy = jp.tile(shape, F16, tag="jy16", bufs=1, name="jy")
            nc.vector.tensor_scalar(out=jy[0:95], in0=t2[0:95], scalar1=0.5,
                                    scalar2=None, op0=Alu.mult)
            nc.sync.dma_start(out=scr['jy'][:], in_=jy[0:95])

            # jz = 0.5*[(dxBy[h] + dxBy[h+1]) - (dyBx0 + dyBx1)], rows 0..95
            dxby = tmp("dxby")
            nc.vector.tensor_tensor(out=dxby[0:96], in0=bym0[0:96], in1=bym1[0:96],
                                    op=Alu.subtract)
            aa = t95("aa")
            nc.vector.tensor_tensor(out=aa[0:96], in0=dxby[0:96, :, 0:95],
                                    in1=dxby[0:96, :, 1:96], op=Alu.add)
            dybx0 = t95("dybx0")
            nc.vector.tensor_tensor(out=dybx0[0:96], in0=bxm0[0:96, :, 0:95],
                                    in1=bxm0[0:96, :, 1:96], op=Alu.subtract)
            dybx1 = t95("dybx1")
            nc.vector.tensor_tensor(out=dybx1[0:96], in0=bxm1[0:96, :, 0:95],
                                    in1=bxm1[0:96, :, 1:96], op=Alu.subtract)
            bb = t95("bb")
            nc.vector.tensor_tensor(out=bb[0:96], in0=dybx0[0:96], in1=dybx1[0:96],
                                    op=Alu.add)
            tt = t95("tt")
            nc.vector.tensor_tensor(out=tt[0:96], in0=aa[0:96], in1=bb[0:96],
                                    op=Alu.subtract)
            jz = jp.tile(sh95, F16, tag="jz16", bufs=1, name="jz")
            nc.vector.tensor_scalar(out=jz[0:96], in0=tt[0:96], scalar1=0.5,
                                    scalar2=None, op0=Alu.mult)
            nc.sync.dma_start(out=scr['jz'][:], in_=jz[0:96])


def _emit_pass1_flux(nc, tc, FX, acc, maskp):
    """cal_div_c_old for both variants; accumulate Sf, Sf2 into acc slots.
    All flux math on partition rows [0:95) (corner W extent)."""
    nc.tc = tc
    shape = [128, 13, 96]
    NR = 95

    with tc.tile_pool(name="flux", bufs=1) as pool:
        T, TS = {}, {}
        for f in ['bxp', 'byp', 'bxt', 'byt', 'bzt', 'z']:
            t = pool.tile(shape, F32, tag=f"T_{f}", bufs=1, name=f"T_{f}")
            nc.sync.dma_start(out=t[0:96], in_=FX[f][:])
            T[f] = t
            s = pool.tile(shape, F32, tag=f"S_{f}", bufs=1, name=f"S_{f}")
            nc.sync.dma_start(out=s[0:95], in_=FX[f][1:96])
            TS[f] = s

        maskT = _make_mask(nc, tc, pool, "maskT", shape, NR, T['bxp'][0:NR],
                           T['byp'][0:NR], T['bxt'][0:NR], T['byt'][0:NR])
        maskS = _make_mask(nc, tc, pool, "maskS", shape, NR, TS['bxp'][0:NR],
                           TS['byp'][0:NR], TS['bxt'][0:NR], TS['byt'][0:NR])
        bxmT = pool.tile(shape, F32, tag="bxmT", bufs=1, name="bxmT")
        nc.vector.tensor_tensor(out=bxmT[0:NR], in0=T['bxt'][0:NR],
                                in1=maskT[0:NR], op=Alu.mult)
        bymT = pool.tile(shape, F32, tag="bymT", bufs=1, name="bymT")
        nc.vector.tensor_tensor(out=bymT[0:NR], in0=T['byt'][0:NR],
                                in1=maskT[0:NR], op=Alu.mult)
        bxmS = pool.tile(shape, F32, tag="bxmS", bufs=1, name="bxmS")
        nc.vector.tensor_tensor(out=bxmS[0:NR], in0=TS['bxt'][0:NR],
                                in1=maskS[0:NR], op=Alu.mult)
        bymS = pool.tile(shape, F32, tag="bymS", bufs=1, name="bymS")
        nc.vector.tensor_tensor(out=bymS[0:NR], in0=TS['byt'][0:NR],
                                in1=maskS[0:NR], op=Alu.mult)
        Tm = {'bx': bxmT, 'by': bymT}
        TSm = {'bx': bxmS, 'by': bymS}

        for (h0, hcf) in chunks_for(95, HCF):
            _emit_flux_chunk(nc, tc, T, TS, Tm, TSm, acc, maskp, h0, hcf, NR)


def _emit_flux_chunk(nc, tc, T, TS, Tm, TSm, acc, maskp, h0, hcf, NR):
    cs = [128, 12, hcf]

    def C(fld, i, j, l):
        base = TS[fld] if l == 1 else T[fld]
        return base[0:NR, i:i + 12, h0 + j:h0 + j + hcf]

    def Cv(variant, xy, i, j, l):
        if variant == 'p':
            return C('bxp' if xy == 'bx' else 'byp', i, j, l)
        base = TSm[xy] if l == 1 else Tm[xy]
        return base[0:NR, i:i + 12, h0 + j:h0 + j + hcf]

    with tc.tile_pool(name=f"fxc_{h0}", bufs=1) as pool:
        def mk(tag, bufs, name, dt=F32):
            return pool.tile(cs, dt, tag=tag, bufs=bufs, name=name)

        def tt(op, a, b, tag, bufs):
            o = mk(tag, bufs, f"{tag}_o")
            nc.vector.tensor_tensor(out=o[0:NR], in0=a, in1=b, op=op)
            return o[0:NR]

        def ts(op, a, s1, s2=None, op2=None, tag="v", bufs=26):
            o = mk(tag, bufs, f"{tag}_s")
            nc.vector.tensor_scalar(out=o[0:NR], in0=a, scalar1=s1, scalar2=s2,
                                    op0=op, op1=op2 if op2 else Alu.bypass)
            return o[0:NR]

        # shared z pieces
        za = {}
        for (i, j) in [(0, 0), (0, 1), (1, 0), (1, 1)]:
            d = tt(Alu.subtract, C('z', i, j, 1), C('z', i, j, 0), "za", 10)
            o = mk("za", 10, "za_abs")
            nc.scalar.activation(out=o[0:NR], in_=d, func=ActF.Abs)
            za[(i, j)] = o[0:NR]
        P1 = tt(Alu.add, za[(1, 0)], za[(1, 1)], "za", 10)
        P0 = tt(Alu.add, za[(0, 0)], za[(0, 1)], "za", 10)
        PH1 = tt(Alu.add, za[(0, 1)], za[(1, 1)], "za", 10)
        PH0 = tt(Alu.add, za[(0, 0)], za[(1, 0)], "za", 10)
        zd01 = tt(Alu.subtract, C('z', 0, 0, 1), C('z', 1, 0, 1), "zt", 9)
        zd11 = tt(Alu.subtract, C('z', 0, 1, 1), C('z', 1, 1, 1), "zt", 9)
        zh11 = tt(Alu.subtract, C('z', 1, 0, 1), C('z', 1, 1, 1), "zt", 9)
        zh01 = tt(Alu.subtract, C('z', 0, 0, 1), C('z', 0, 1, 1), "zt", 9)
        zd00 = tt(Alu.subtract, C('z', 0, 0, 0), C('z', 1, 0, 0), "zt", 9)
        zdd10 = tt(Alu.subtract, C('z', 0, 1, 0), C('z', 1, 1, 0), "zt", 9)
        zhh10 = tt(Alu.subtract, C('z', 1, 0, 0), C('z', 1, 1, 0), "zt", 9)
        zh00 = tt(Alu.subtract, C('z', 0, 0, 0), C('z', 0, 1, 0), "zt", 9)

        def sum_corners(get, corners, tag, bufs):
            o = tt(Alu.add, get(*corners[0]), get(*corners[1]), tag, bufs)
            for c in corners[2:]:
                o = tt(Alu.add, o, get(*c), tag, bufs)
            return o

        def Cz(i, j, l):
            return C('bzt', i, j, l)

        t1a = sum_corners(Cz, [(0, 0, 1), (1, 0, 1), (1, 1, 1)], "bz", 11)
        t1b = sum_corners(Cz, [(0, 0, 1), (1, 1, 1), (0, 1, 1)], "bz", 11)
        bzs1 = tt(Alu.add, t1a, t1b, "bz", 11)
        t0a = sum_corners(Cz, [(0, 0, 0), (1, 0, 0), (1, 1, 0)], "bz", 11)
        t0b = sum_corners(Cz, [(0, 0, 0), (1, 1, 0), (0, 1, 0)], "bz", 11)
        bzs0 = tt(Alu.add, t0a, t0b, "bz", 11)
        bzdiff = tt(Alu.subtract, bzs1, bzs0, "bz", 11)
        bz8 = sum_corners(Cz, [(i, j, l) for i in (0, 1) for j in (0, 1)
                               for l in (0, 1)], "bz", 11)
        bz8s = ts(Alu.mult, bz8, 0.125, tag="bz", bufs=11)
        bz8sq = tt(Alu.mult, bz8s, bz8s, "bz", 11)

        for variant in ['p', 't']:
            def Cx(i, j, l, _v=variant):
                return Cv(_v, 'bx', i, j, l)

            def Cy(i, j, l, _v=variant):
                return Cv(_v, 'by', i, j, l)

            V = ("v", 26)
            bxs1 = sum_corners(Cx, [(1, 0, 0), (1, 1, 0), (1, 0, 1), (1, 1, 1)], *V)
            bxs0 = sum_corners(Cx, [(0, 0, 0), (0, 1, 0), (0, 0, 1), (0, 1, 1)], *V)
            bysj1 = sum_corners(Cy, [(0, 1, 0), (1, 1, 0), (0, 1, 1), (1, 1, 1)], *V)
            bysj0 = sum_corners(Cy, [(0, 0, 0), (1, 0, 0), (0, 0, 1), (1, 0, 1)], *V)
            x1a = sum_corners(Cx, [(0, 0, 1), (1, 0, 1), (1, 1, 1)], *V)
            x1b = sum_corners(Cx, [(0, 0, 1), (0, 1, 1), (1, 1, 1)], *V)
            x0a = sum_corners(Cx, [(0, 0, 0), (1, 0, 0), (1, 1, 0)], *V)
            x0b = sum_corners(Cx, [(0, 0, 0), (0, 1, 0), (1, 1, 0)], *V)
            y1a = sum_corners(Cy, [(0, 0, 1), (1, 0, 1), (1, 1, 1)], *V)
            y1b = sum_corners(Cy, [(0, 0, 1), (0, 1, 1), (1, 1, 1)], *V)
            y0a = sum_corners(Cy, [(0, 0, 0), (1, 0, 0), (1, 1, 0)], *V)
            y0b = sum_corners(Cy, [(0, 0, 0), (0, 1, 0), (1, 1, 0)], *V)

            g1 = tt(Alu.mult, bxs1, P1, *V)
            g2 = tt(Alu.mult, bxs0, P0, *V)
            gA = tt(Alu.subtract, g1, g2, *V)
            g3 = tt(Alu.mult, bysj1, PH1, *V)
            g4 = tt(Alu.mult, bysj0, PH0, *V)
            gB = tt(Alu.add, gA, g3, *V)
            gC = tt(Alu.subtract, gB, g4, *V)

            h1 = tt(Alu.mult, x1a, zd01, *V)
            h2 = tt(Alu.mult, x1b, zd11, *V)
            hA = tt(Alu.add, h1, h2, *V)
            h3 = tt(Alu.mult, y1a, zh11, *V)
            h4 = tt(Alu.mult, y1b, zh01, *V)
            hB = tt(Alu.add, h3, h4, *V)
            hAB = tt(Alu.add, hA, hB, *V)
            h5 = tt(Alu.mult, x0a, zd00, *V)
            h6 = tt(Alu.mult, x0b, zdd10, *V)
            hC = tt(Alu.add, h5, h6, *V)
            h7 = tt(Alu.mult, y0a, zhh10, *V)
            h8 = tt(Alu.mult, y0b, zh00, *V)
            hD = tt(Alu.add, h7, h8, *V)
            hCD = tt(Alu.add, hC, hD, *V)
            hdiff = tt(Alu.subtract, hAB, hCD, *V)
            hfull = tt(Alu.add, hdiff, bzdiff, *V)

            gs = ts(Alu.mult, gC, 0.125, tag="v", bufs=26)
            hs = ts(Alu.mult, hfull, 1.0 / 6.0, tag="v", bufs=26)
            flux = tt(Alu.add, gs, hs, *V)

            res2 = mk("vf32", 8, "res2", dt=F32)
            nc.vector.tensor_tensor(out=res2[0:NR], in0=flux, in1=flux,
                                    op=Alu.mult)
            res4 = mk("vf32", 8, "res4", dt=F32)
            nc.vector.tensor_tensor(out=res4[0:NR], in0=res2[0:NR],
                                    in1=res2[0:NR], op=Alu.mult)
            res4 = res4[0:NR]
            bx8 = tt(Alu.add, bxs1, bxs0, *V)
            bx8s = ts(Alu.mult, bx8, 0.125, tag="v", bufs=26)
            bx8sq = tt(Alu.mult, bx8s, bx8s, *V)
            by8 = tt(Alu.add, bysj1, bysj0, *V)
            by8s = ts(Alu.mult, by8, 0.125, tag="v", bufs=26)
            by8sq = tt(Alu.mult, by8s, by8s, *V)
            ab1 = tt(Alu.add, bx8sq, by8sq, *V)
            ab2 = tt(Alu.add, ab1, bz8sq, *V)
            aveb = mk("vf32", 8, "aveb", dt=F32)
            nc.vector.tensor_scalar(out=aveb[0:NR], in0=ab2, scalar1=1e-8,
                                    scalar2=None, op0=Alu.add)
            # divide is unsupported: reciprocal (ACT) + one Newton step
            rcp = mk("vf32", 8, "rcp", dt=F32)
            nc.vector.reciprocal(out=rcp[0:NR], in_=aveb[0:NR])
            ar = mk("vf32", 8, "ar", dt=F32)
            nc.vector.tensor_tensor(out=ar[0:NR], in0=aveb[0:NR], in1=rcp[0:NR],
                                    op=Alu.mult)
            two_m = mk("vf32", 8, "two_m", dt=F32)
            nc.vector.tensor_scalar(out=two_m[0:NR], in0=ar[0:NR], scalar1=-1.0,
                                    scalar2=2.0, op0=Alu.mult, op1=Alu.add)
            rcp2 = mk("vf32", 8, "rcp2", dt=F32)
            nc.vector.tensor_tensor(out=rcp2[0:NR], in0=rcp[0:NR],
                                    in1=two_m[0:NR], op=Alu.mult)
            flx1 = mk("vf32", 8, "flx1", dt=F32)
            nc.vector.tensor_tensor(out=flx1[0:NR], in0=res4,
                                    in1=rcp2[0:NR], op=Alu.mult)
            flx1 = flx1[0:NR]

            _acc_masked_sums(nc, pool, acc, maskp, flx1, cs, NR,
                             SLOT[f'f_{variant}'], SLOT[f'f2_{variant}'],
                             nplanes=12, mask_last=True)


def _acc_masked_sums(nc, pool, acc, maskp, fld, fshape, NR, slot1, slot2, nplanes,
                     mask_last):
    """acc[slot1] += sum(fld), acc[slot2] += sum(fld^2); optional mask on the
    last plane. fld: AP [NR, nplanes, X]."""
    sq = pool.tile(fshape, F32, tag="sq", bufs=2, name="sq")
    nc.scalar.activation(out=sq[0:NR], in_=fld, func=ActF.Square)

    def r(name):
        return pool.tile([128, 1], F32, tag="r", bufs=8, name=name)

    for (slot, fsrc) in [(slot1, fld), (slot2, sq[0:NR])]:
        ra = r("ra")
        nc.vector.tensor_reduce(out=ra[0:NR], in_=fsrc[:, 0:nplanes - 1, :],
                                axis=mybir.AxisListType.XY, op=Alu.add)
        rb = r("rb")
        nc.vector.tensor_reduce(out=rb[0:NR], in_=fsrc[:, nplanes - 1:nplanes, :],
                                axis=mybir.AxisListType.XY, op=Alu.add)
        if mask_last:
            rbm = r("rbm")
            nc.vector.tensor_tensor(out=rbm[0:NR], in0=rb[0:NR], in1=maskp[0:NR],
                                    op=Alu.mult)
            rb = rbm
        rs = r("rs")
        nc.vector.tensor_tensor(out=rs[0:NR], in0=ra[0:NR], in1=rb[0:NR], op=Alu.add)
        nc.vector.tensor_tensor(out=acc[0:NR, slot:slot + 1],
                                in0=acc[0:NR, slot:slot + 1],
                                in1=rs[0:NR], op=Alu.add)


_ENG_CTR = [0]


def _med_eng(nc):
    """Weighted engine rotation for the median CE networks."""
    _ENG_CTR[0] += 1
    if GP_MOD and _ENG_CTR[0] % GP_MOD == 0:
        return nc.gpsimd
    return nc.vector


def _emit_pass2_medians(nc, tc, AH, scr, acc):
    nc.tc = tc
    src_map = {
        'bxp': (AH['bxp'], 96, 96),
        'byp': (AH['byp'], 96, 96),
        'bxm': (scr['bxm'], 96, 96),
        'bym': (scr['bym'], 96, 96),
        'jx': (scr['jx'], 95, 95),
        'jy': (scr['jy'], 95, 96),
        'jz': (scr['jz'], 96, 95),
    }
    with tc.tile_pool(name="medglobal", bufs=1) as pool:
        for fname in FILTER_ORDER:
            dram, Wext, Hext = src_map[fname]
            mask_last = fname in ('jy', 'jz')
            _emit_one_median(nc, tc, pool, fname, dram, Wext, Hext, acc,
                             mask_last)


def _emit_one_median(nc, tc, pool, fname, dram, Wext, Hext, acc, mask_last):
    He = Hext + 4
    NR = Wext
    # s25f rotates per-field within one shared 25-buffer set; X tiles are
    # double-buffered so the next field's loads overlap this field's compute
    s25f = [pool.tile([128, DIN, 96], F16, tag="s25f", bufs=25,
                      name=f"s25f_{i}") for i in range(25)]
    X = []
    for k in range(5):
        xk = pool.tile([128, DIN, 100], F16, tag=f"X{k}",
                       bufs=2 if k == 2 else 1, name=f"X{fname}{k}")
        p_lo = max(0, 2 - k)
        p_hi = min(Wext, Wext + 2 - k)
        nc.sync.dma_start(out=xk[p_lo:p_hi, :, 2:2 + Hext],
                          in_=dram[p_lo + k - 2:p_hi + k - 2, :, :])
        for p in list(range(0, p_lo)) + list(range(p_hi, Wext)):
            w = refl(p + k - 2, Wext)
            nc.sync.dma_start(out=xk[p:p + 1, :, 2:2 + Hext],
                              in_=dram[w:w + 1, :, :])
        for (dst, srcc) in [(0, 4), (1, 3), (2 + Hext, Hext),
                            (3 + Hext, Hext - 1)]:
            nc.sync.dma_start(out=xk[0:Wext, :, dst:dst + 1],
                              in_=xk[0:Wext, :, srcc:srcc + 1])
        X.append(xk)

    for (h0, hc) in chunks_for(Hext, HC):
        _emit_phase2_chunk(nc, tc, X, h0, hc, NR, s25f)

    for (h0, hc) in chunks_for(Hext, HC3):
        _emit_phase3_chunk(nc, tc, s25f, X[2], h0, hc, NR, acc, fname,
                           mask_last)


def _emit_phase2_chunk(nc, tc, X, h0, hc, NR, s25f):
    hc4 = hc + 4
    with tc.tile_pool(name=f"mp2_{h0}", bufs=1) as p2:
        wires = {i: X[i][0:NR, :, h0:h0 + hc4] for i in range(5)}
        emit_ces_nr(nc, p2, "s", 9, [128, DIN, hc4], NR, wires,
                    _prune(SORT5, set(range(5))))
        s = [wires[i] for i in range(5)]

        ces, order = merge_net(5, 5)
        w = {}
        for i in range(5):
            w[i] = s[i][:, :, 0:hc + 3]
            w[5 + i] = s[i][:, :, 1:hc4]
        emit_ces_nr(nc, p2, "p10", 12, [128, DIN, hc + 3], NR, w, ces)
        P10 = [w[o] for o in order]

        ces, order = merge_net(10, 5)
        w = {}
        for i in range(10):
            w[i] = P10[i][:, :, 0:hc + 2]
        for i in range(5):
            w[10 + i] = s[i][:, :, 2:hc4]
        emit_ces_nr(nc, p2, "t15", 17, [128, DIN, hc + 2], NR, w, ces)
        T15 = [w[o] for o in order]

        ces, order = merge_net(15, 10)
        w = {}
        for i in range(15):
            w[i] = T15[i][:, :, 2:hc + 2]
        for i in range(10):
            w[15 + i] = P10[i][:, :, 0:hc]
        # final writers land directly in the persistent full-H S25 slices
        final_dst = {order[r]: s25f[r][0:NR, :, h0:h0 + hc] for r in range(25)}
        emit_ces_nr(nc, p2, "s25", 26, [128, DIN, hc], NR, w, ces,
                    final_dst=final_dst)


def _emit_phase3_chunk(nc, tc, s25f, x2, h0, hc, NR, acc, fname, mask_last):
    S25 = [s25f[i][0:NR, :, h0:h0 + hc] for i in range(25)]
    X2 = x2

    with tc.tile_pool(name=f"mp3_{h0}", bufs=1) as p3:
        if True:
            ces, order = merge_net(25, 25)
            w = {}
            for i in range(25):
                w[i] = S25[i][:, 1:14:2, :]
                w[25 + i] = S25[i][:, 2:15:2, :]
            emit_ces_nr(nc, p3, "m2", 52, [128, 7, hc], NR, w, ces)
            M2 = [w[o] for o in order]

            ces, order = merge_net(50, 50, needed_ranks=range(37, 63))
            w = {}
            for i in range(50):
                w[i] = M2[i][:, 0:6, :]
                w[50 + i] = M2[i][:, 1:7, :]
            emit_ces_nr(nc, p3, "q", 31, [128, 6, hc], NR, w, ces)
            Q = {r: w[order[r]] for r in range(37, 63)}

            meds = []
            for par, sel in [(0, slice(0, 11, 2)), (1, slice(5, 16, 2))]:
                Sside = [S25[i][:, sel, :] for i in range(25)]
                accw = Q[62]
                for j in range(1, 26):
                    t = p3.tile([128, 6, hc], F16, tag="sel", bufs=6, name="sel_t")
                    _med_eng(nc).tensor_tensor(out=t[0:NR], in0=Q[62 - j],
                                               in1=Sside[j - 1], op=Alu.max)
                    t2 = p3.tile([128, 6, hc], F16, tag="sel", bufs=6, name="sel_m")
                    _med_eng(nc).tensor_tensor(out=t2[0:NR], in0=accw, in1=t[0:NR],
                                               op=Alu.min)
                    accw = t2[0:NR]
                meds.append(accw)

            for par, med in enumerate(meds):
                xs = X2[0:NR, 2 + par:14 + par:2, h0 + 2:h0 + 2 + hc]
                d = p3.tile([128, 6, hc], F16, tag="sel", bufs=6, name="d")
                _med_eng(nc).tensor_tensor(out=d[0:NR], in0=med, in1=xs,
                                           op=Alu.subtract)
                d2 = p3.tile([128, 6, hc], F16, tag="sel", bufs=6, name="d2")
                if par == 1 and mask_last:
                    cm = MED_SLOT[(fname, h0, par, 'm')]
                    nc.scalar.activation(out=d2[0:NR, 0:5, :], in_=d[0:NR, 0:5, :],
                                         func=ActF.Square,
                                         accum_out=acc[0:NR, cm:cm + 1])
                    cl = MED_SLOT[(fname, h0, par, 'l')]
                    nc.scalar.activation(out=d2[0:NR, 5:6, :], in_=d[0:NR, 5:6, :],
                                         func=ActF.Square,
                                         accum_out=acc[0:NR, cl:cl + 1])
                else:
                    cm = MED_SLOT[(fname, h0, par, 'm')]
                    nc.scalar.activation(out=d2[0:NR], in_=d[0:NR],
                                         func=ActF.Square,
                                         accum_out=acc[0:NR, cm:cm + 1])


def emit_ces_nr(nc, pool, tag, bufs, shape, NR, wires, ces, final_dst=None):
    """SSA compare-exchange emission on partition rows [0:NR). Wires listed in
    final_dst have their LAST write redirected to the given AP (which must
    already be partition-sliced to [0:NR))."""
    last_write = {}
    if final_dst:
        for idx, (u, v, nm, nM) in enumerate(ces):
            if nm:
                last_write[u] = idx
            if nM:
                last_write[v] = idx
    for idx, (u, v, nm, nM) in enumerate(ces):
        a, b = wires[u], wires[v]
        dmn = dmx = None
        if final_dst:
            if nm and u in final_dst and last_write.get(u) == idx:
                dmn = final_dst[u]
            if nM and v in final_dst and last_write.get(v) == idx:
                dmx = final_dst[v]
        if nm:
            if dmn is None:
                tmn = pool.tile(shape, F16, tag=tag, bufs=bufs, name=f"{tag}_mn")
                dmn = tmn[0:NR]
            _med_eng(nc).tensor_tensor(out=dmn, in0=a, in1=b, op=Alu.min)
        if nM:
            if dmx is None:
                tmx = pool.tile(shape, F16, tag=tag, bufs=bufs, name=f"{tag}_mx")
                dmx = tmx[0:NR]
            _med_eng(nc).tensor_tensor(out=dmx, in0=a, in1=b, op=Alu.max)
        if nm:
            wires[u] = dmn
        if nM:
            wires[v] = dmx


# ---------------------------------------------------------------------------
# host side
# ---------------------------------------------------------------------------

def _arrange(f, idx):
    """f: [D, H, W] -> [W, len(idx), H] contiguous."""
    return np.ascontiguousarray(np.asarray(f)[np.asarray(idx)].transpose(2, 0, 1))


def make_in_maps(pred_b, pred_z, targets):
    pb = np.asarray(pred_b, dtype=np.float32)[0]
    pz = np.asarray(pred_z, dtype=np.float32)[0, 0]
    tg = np.asarray(targets, dtype=np.float32)[0]
    fields = {
        'bxp': pb[0], 'byp': pb[1], 'bzp': pb[2],
        'bxt': tg[0], 'byt': tg[1], 'bzt': tg[2],
        'z': pz,
    }
    in_maps = []
    for c in range(NCORES):
        m = {}
        a_idx = [refl(12 * c - 2 + s, 96) for s in range(DIN)]
        jg = [refl(12 * c - 2 + s, 95) for s in range(DIN)]
        j1_idx = [g + 1 for g in jg]
        fx_idx = [min(12 * c + s, 95) for s in range(13)]
        for f in ['bxt', 'byt', 'bxp', 'byp', 'bzp']:
            m[f"A_{f}"] = _arrange(fields[f], a_idx)
            m[f"J0_{f}"] = _arrange(fields[f], jg)
            m[f"J1_{f}"] = _arrange(fields[f], j1_idx)
        for f in ['bxp', 'byp', 'bxt', 'byt', 'bzt', 'z']:
            m[f"Fx_{f}"] = _arrange(fields[f], fx_idx)
        m["Ah_bxp"] = m["A_bxp"].astype(np.float16)
        m["Ah_byp"] = m["A_byp"].astype(np.float16)
        mp = np.zeros((128, 1), dtype=np.float32)
        mp[:] = 0.0 if c == NCORES - 1 else 1.0
        m["maskp"] = mp
        in_maps.append(m)
    return in_maps


def combine(outs):
    """outs: list of 8 arrays [128, NSLOT] -> 6-scalar loss tuple."""
    def tot(slot, we):
        return float(sum(np.asarray(o)[:we, slot].astype(np.float64).sum()
                         for o in outs))

    def med_tot(fname, we):
        s = 0.0
        for (fn, h0, par, blk), col in MED_SLOT.items():
            if fn != fname:
                continue
            for ci, o in enumerate(outs):
                if blk == 'l' and ci == NCORES - 1:
                    continue
                s += float(np.asarray(o)[:we, col].astype(np.float64).sum())
        return s

    N95 = 95.0 ** 3
    s_fp = tot(SLOT['f_p'], 95)
    s_f2p = tot(SLOT['f2_p'], 95)
    s_ft = tot(SLOT['f_t'], 95)
    s_f2t = tot(SLOT['f2_t'], 95)
    loss_div_p = s_fp / N95
    std_p = s_f2p / N95 - loss_div_p ** 2
    loss_div_t = s_ft / N95
    std_t = s_f2t / N95 - loss_div_t ** 2
    loss_j = (med_tot('jx', 95) / (96 * 95 * 95)
              + med_tot('jy', 95) / (95 * 96 * 95)
              + med_tot('jz', 96) / (95 * 95 * 96))
    N96 = 96.0 ** 3
    loss_b = (med_tot('bxm', 96) + med_tot('bym', 96)
              + med_tot('bxp', 96) + med_tot('byp', 96)) / N96
    return (np.float32(loss_div_p), np.float32(std_p), np.float32(loss_div_t),
            np.float32(std_t), np.float32(loss_j), np.float32(loss_b))


_NC_CACHE = None


def get_program():
    """Program for hardware execution (multi-wait legalized)."""
    global _NC_CACHE
    if _NC_CACHE is None:
        nc = build_program()
        _legalize_multiwaits(nc)
        _NC_CACHE = nc
    return _NC_CACHE


def kernel(pred_b, pred_z, targets, iepoch=None, epoch_max=None):
    nc = get_program()
    in_maps = make_in_maps(pred_b, pred_z, targets)
    res = run_bass_kernel_spmd(nc, in_maps, list(range(NCORES)))
    outs = [res.results[i]["out"] for i in range(NCORES)]
    return combine(outs)



# revision 34
# speedup vs baseline: 1.0115x; 1.0115x over previous
"""Trainium2 Bass kernel for nn_CustomLoss_Z_B_25031069401264.

Computes the 6-scalar custom loss (divergence fluxes + variances, 5x5x5
median-filter smoothness losses) for inputs pred_b [1,3,96,96,96],
pred_z [1,1,96,96,96], targets [1,3,96,96,96].

Strategy:
  - D axis sharded across 8 cores (12 output planes each). Host pre-slices
    overlapping input slabs with all D-reflects resolved, so the SPMD
    program is identical on every core.
  - On-chip layout: W on the partition axis, (D-plane, H) in the free dim.
    W-shifts are materialized as 5 DMA-aligned copies; H and D shifts are
    free-dim AP offsets.
  - Exact medians via shared sorting networks: sort5 along W -> sorted-25
    per voxel (Batcher merges along H) -> paired D-phase: merge(25,25) at
    odd positions, pruned merge(50,50) to ranks 37..62 shared by two
    consecutive output planes, then a 26-term min/max rank select.
  - Each core emits per-partition partial sums [128,16]; host combines in
    float64 and returns the 6 scalars.
"""

import numpy as np
from concourse import bass, mybir
from concourse.tile import TileContext
from concourse.bass_utils import run_bass_kernel_spmd

F32 = mybir.dt.float32
F16 = mybir.dt.float16
Alu = mybir.AluOpType
ActF = mybir.ActivationFunctionType

NCORES = 8
DS = 12          # output D planes per core
DIN = 16         # median field slab planes per core: [12c-2, 12c+14)
HC = 48          # H chunk size for the median phase-2 (sorted-25)
HC3 = 96         # H chunk size for the median phase-3 (D-selection)
HCF = 32         # H chunk size for the flux pass
GP_MOD = 0       # every GP_MOD-th median CE op goes to GpSimd (0 = disable)

# ---------------------------------------------------------------------------
# sorting / merge networks
# ---------------------------------------------------------------------------
SORT5 = [(0, 1), (3, 4), (2, 4), (2, 3), (1, 4), (0, 3), (0, 2), (1, 3), (1, 2)]


def _oem_merge(A, B, ces):
    if not A:
        return list(B)
    if not B:
        return list(A)
    if len(A) == 1 and len(B) == 1:
        ces.append((A[0], B[0]))
        return [A[0], B[0]]
    E = _oem_merge(A[0::2], B[0::2], ces)
    O = _oem_merge(A[1::2], B[1::2], ces)
    out = [E[0]]
    i = 0
    while i < len(O) and i + 1 < len(E):
        ces.append((O[i], E[i + 1]))
        out.append(O[i])
        out.append(E[i + 1])
        i += 1
    out.extend(O[i:])
    out.extend(E[i + 1:])
    return out


def _prune(ces, needed_wires):
    needed = set(needed_wires)
    kept = []
    for (u, v) in reversed(ces):
        nm, nM = u in needed, v in needed
        if not (nm or nM):
            continue
        kept.append((u, v, nm, nM))
        needed.add(u)
        needed.add(v)
    kept.reverse()
    return kept


_MERGE_CACHE = {}


def merge_net(m, n, needed_ranks=None):
    """Returns (pruned_ces, order). Wires 0..m-1 = sorted A, m..m+n-1 = sorted B."""
    key = (m, n, tuple(needed_ranks) if needed_ranks is not None else None)
    if key in _MERGE_CACHE:
        return _MERGE_CACHE[key]
    ces = []
    order = _oem_merge(list(range(m)), list(range(m, m + n)), ces)
    if needed_ranks is None:
        pruned = _prune(ces, set(range(m + n)))
    else:
        pruned = _prune(ces, {order[r] for r in needed_ranks})
    _MERGE_CACHE[key] = (pruned, order)
    return pruned, order


def peak_live(ces, n_wires, final_needed=None):
    version = {i: ('ext', i) for i in range(n_wires)}
    reads = {}
    tid = 0
    creations = []
    for step, (u, v, nm, nM) in enumerate(ces):
        for w in (u, v):
            t = version[w]
            if t[0] == 'tile':
                reads[t[1]] = step
        if nm:
            creations.append((step, tid))
            version[u] = ('tile', tid)
            tid += 1
        if nM:
            creations.append((step, tid))
            version[v] = ('tile', tid)
            tid += 1
    end = len(ces)
    finals = set()
    fw = final_needed if final_needed is not None else range(n_wires)
    for wdx in fw:
        t = version[wdx]
        if t[0] == 'tile':
            finals.add(t[1])
    events = []
    for (cs, t) in creations:
        last = end if t in finals else reads.get(t, cs)
        events.append((cs, 1))
        events.append((last + 0.5, -1))
    events.sort()
    cur = peak = 0
    for (_, d) in events:
        cur += d
        peak = max(peak, cur)
    return peak


# ---------------------------------------------------------------------------
# emitter helpers
# ---------------------------------------------------------------------------

def emit_ces(nc, pool, tag, bufs, shape, wires, ces):
    """SSA compare-exchange emission. wires: dict idx -> AP (pre-sliced to a
    common extent == shape[1:]). Updates wires in place."""
    for (u, v, nm, nM) in ces:
        a, b = wires[u], wires[v]
        if nm:
            tmn = pool.tile(shape, F32, tag=tag, bufs=bufs, name=f"{tag}_mn")
            nc.vector.tensor_tensor(out=tmn[:], in0=a, in1=b, op=Alu.min)
        if nM:
            tmx = pool.tile(shape, F32, tag=tag, bufs=bufs, name=f"{tag}_mx")
            nc.vector.tensor_tensor(out=tmx[:], in0=a, in1=b, op=Alu.max)
        if nm:
            wires[u] = tmn[:]
        if nM:
            wires[v] = tmx[:]


def refl(d, n):
    if d < 0:
        return -d
    if d >= n:
        return 2 * (n - 1) - d
    return d


FILTER_ORDER = ['bxp', 'byp', 'bxm', 'bym', 'jx', 'jy', 'jz']
SLOT = {'f_p': 0, 'f2_p': 1, 'f_t': 2, 'f2_t': 3}
FIELD_HEXT = {'bxp': 96, 'byp': 96, 'bxm': 96, 'bym': 96,
              'jx': 95, 'jy': 96, 'jz': 95}


def chunks_for(Hext, hc):
    out = []
    h0 = 0
    while h0 < Hext:
        out.append((h0, min(hc, Hext - h0)))
        h0 += hc
    return out


def _build_med_slots():
    """Column map for the per-(field, chunk, par, block) ACT accumulators.
    block 'l' (last D plane of par 1) is split out for jy/jz so the host can
    drop core 7's out-of-range plane."""
    slots = {}
    col = 4
    for fname in FILTER_ORDER:
        for (h0, _) in chunks_for(FIELD_HEXT[fname], HC3):
            for par in (0, 1):
                blks = ('m', 'l') if (par == 1 and fname in ('jy', 'jz')) \
                    else ('m',)
                for blk in blks:
                    slots[(fname, h0, par, blk)] = col
                    col += 1
    return slots, col


MED_SLOT, _NCOL = _build_med_slots()


def _build_flux_slots(col):
    slots = {}
    for variant in ('p', 't'):
        for (h0, _) in chunks_for(95, HCF):
            for comp in ('f', 'f2'):
                for blk in ('m', 'l'):
                    slots[(variant, h0, comp, blk)] = col
                    col += 1
    return slots, col


FLUX_SLOT, _NCOL2 = _build_flux_slots(_NCOL)
NSLOT = 64
assert _NCOL2 <= NSLOT


# ---------------------------------------------------------------------------
# program builder (SPMD; identical for all cores)
# ---------------------------------------------------------------------------

def build_program():
    nc = bass.Bass()

    A = {f: nc.declare_dram_parameter(f"A_{f}", [96, DIN, 96], F16, isOutput=False)
         for f in ['bxt', 'byt', 'bxp', 'byp', 'bzp']}
    J0 = {f: nc.declare_dram_parameter(f"J0_{f}", [96, DIN, 96], F16, isOutput=False)
          for f in ['bxt', 'byt', 'bxp', 'byp', 'bzp']}
    J1 = {f: nc.declare_dram_parameter(f"J1_{f}", [96, DIN, 96], F16, isOutput=False)
          for f in ['bxt', 'byt', 'bxp', 'byp', 'bzp']}
    FX = {f: nc.declare_dram_parameter(f"Fx_{f}", [96, 13, 96], F32, isOutput=False)
          for f in ['bxp', 'byp', 'bxt', 'byt', 'bzt', 'z']}
    AH = {f: nc.declare_dram_parameter(f"Ah_{f}", [96, DIN, 96], F16, isOutput=False)
          for f in ['bxp', 'byp']}
    maskp_ext = nc.declare_dram_parameter("maskp", [128, 1], F32, isOutput=False)
    out_ext = nc.declare_dram_parameter("out", [128, NSLOT], F32, isOutput=True)

    scr = {
        'bxm': nc.dram_tensor("scr_bxm", [96, DIN, 96], F16),
        'bym': nc.dram_tensor("scr_bym", [96, DIN, 96], F16),
        'jx': nc.dram_tensor("scr_jx", [95, DIN, 95], F16),
        'jy': nc.dram_tensor("scr_jy", [95, DIN, 96], F16),
        'jz': nc.dram_tensor("scr_jz", [96, DIN, 95], F16),
    }

    with TileContext(nc) as tc:
        with tc.tile_pool(name="acc", bufs=1) as accpool:
            acc = accpool.tile([128, NSLOT], F32, name="acc")
            nc.vector.memset(acc[:], 0.0)
            maskp = accpool.tile([128, 1], F32, name="maskp_t")
            nc.sync.dma_start(out=maskp[:], in_=maskp_ext[:])

            _emit_pass1_fields(nc, tc, A, J0, J1, scr)
            _emit_pass1_flux(nc, tc, FX, acc, maskp)
            _emit_pass2_medians(nc, tc, AH, scr, acc)

            nc.sync.dma_start(out=out_ext[:], in_=acc[:])
    return nc


def _legalize_multiwaits(nc):
    """This walrus build only supports ONE sync-wait per instruction. Move
    excess waits onto injected same-engine NoOps (sequencer stalls there,
    preserving ordering exactly)."""
    ctr = 0
    for fn in nc.m.functions:
        for bb in fn.blocks:
            insts = bb.instructions
            new = []
            changed = False
            for inst in insts:
                si = inst.sync_info
                if si is not None and si.on_wait and len(si.on_wait) > 1:
                    waits = list(si.on_wait)
                    for w in waits[:-1]:
                        nop = mybir.InstNoOp(name=f"waitnop_{ctr}")
                        ctr += 1
                        nop.engine = inst.engine
                        nop.sync_info = mybir.SyncInfo(on_wait=[w], on_update=[])
                        new.append(nop)
                    inst.sync_info = mybir.SyncInfo(on_wait=[waits[-1]],
                                                    on_update=list(si.on_update))
                    changed = True
                new.append(inst)
            if changed:
                bb.instructions = new
    return nc


def _make_mask(nc, tc, pool_persist, persist_tag, shape, nr, bxp, byp, bxt, byt,
               dt=F32):
    """mask = 2*(bxp*bxt + byp*byt > 0) - 1 into a persistent tile, using a
    transient inner pool. All ops on partition rows [0:nr)."""
    mk = pool_persist.tile(shape, dt, tag=persist_tag, bufs=2, name=persist_tag)
    with nc.tc.tile_pool(name=f"mk_{persist_tag}", bufs=1) as mp:
        t1 = mp.tile(shape, dt, tag="mt", bufs=5, name="mt_1")
        nc.vector.tensor_tensor(out=t1[0:nr], in0=bxp, in1=bxt, op=Alu.mult)
        t2 = mp.tile(shape, dt, tag="mt", bufs=5, name="mt_2")
        nc.vector.tensor_tensor(out=t2[0:nr], in0=byp, in1=byt, op=Alu.mult)
        t3 = mp.tile(shape, dt, tag="mt", bufs=5, name="mt_3")
        nc.vector.tensor_tensor(out=t3[0:nr], in0=t1[0:nr], in1=t2[0:nr], op=Alu.add)
        m = mp.tile(shape, dt, tag="mt", bufs=5, name="mt_4")
        nc.vector.tensor_scalar(out=m[0:nr], in0=t3[0:nr], scalar1=0.0, scalar2=None,
                                op0=Alu.is_gt)
        nc.vector.tensor_scalar(out=mk[0:nr], in0=m[0:nr], scalar1=2.0, scalar2=-1.0,
                                op0=Alu.mult, op1=Alu.add)
    return mk


def _emit_pass1_fields(nc, tc, A, J0, J1, scr):
    """Compute bxm, bym (A-arranged) and jx, jy, jz; write DRAM scratch."""
    nc.tc = tc
    shape = [128, DIN, 96]
    sh95 = [128, DIN, 95]

    with tc.tile_pool(name="p1a", bufs=1) as pool:
        ta = {}
        for f in ['bxt', 'byt', 'bxp', 'byp', 'bzp']:
            t = pool.tile(shape, F16, tag=f"A_{f}", bufs=1, name=f"tA_{f}")
            nc.sync.dma_start(out=t[0:96], in_=A[f][:])
            ta[f] = t
        maskA = _make_mask(nc, tc, pool, "maskA", shape, 96, ta['bxp'][0:96],
                           ta['byp'][0:96], ta['bxt'][0:96], ta['byt'][0:96],
                           dt=F16)
        bxmA = pool.tile(shape, F16, tag="bxmA", bufs=1, name="bxmA")
        nc.vector.tensor_tensor(out=bxmA[0:96], in0=ta['bxt'][0:96],
                                in1=maskA[0:96], op=Alu.mult)
        bymA = pool.tile(shape, F16, tag="bymA", bufs=1, name="bymA")
        nc.vector.tensor_tensor(out=bymA[0:96], in0=ta['byt'][0:96],
                                in1=maskA[0:96], op=Alu.mult)
        nc.sync.dma_start(out=scr['bxm'][:], in_=bxmA[0:96])
        nc.sync.dma_start(out=scr['bym'][:], in_=bymA[0:96])

        # jx = 0.5*[(dyBz + dyBz_s) - (dzBy_h + dzBy_h1)], valid rows 0..94
        with tc.tile_pool(name="p1a_jx", bufs=1) as jp:
            bzpS = jp.tile(shape, F16, tag="tmp", bufs=4, name="bzpS")
            nc.sync.dma_start(out=bzpS[0:95], in_=ta['bzp'][1:96])
            bymS = jp.tile(shape, F16, tag="tmp", bufs=4, name="bymS")
            nc.sync.dma_start(out=bymS[0:95], in_=bymA[1:96])

            def t95(name):
                return jp.tile(sh95, F16, tag="t95", bufs=5, name=name)

            dy0 = t95("dy0")
            nc.vector.tensor_tensor(out=dy0[0:95], in0=ta['bzp'][0:95, :, 0:95],
                                    in1=ta['bzp'][0:95, :, 1:96], op=Alu.subtract)
            dy1 = t95("dy1")
            nc.vector.tensor_tensor(out=dy1[0:95], in0=bzpS[0:95, :, 0:95],
                                    in1=bzpS[0:95, :, 1:96], op=Alu.subtract)
            u = t95("u")
            nc.vector.tensor_tensor(out=u[0:95], in0=dy0[0:95], in1=dy1[0:95],
                                    op=Alu.add)
            dzby = jp.tile(shape, F16, tag="tmp", bufs=4, name="dzby")
            nc.vector.tensor_tensor(out=dzby[0:95], in0=bymA[0:95], in1=bymS[0:95],
                                    op=Alu.subtract)
            v = t95("v")
            nc.vector.tensor_tensor(out=v[0:95], in0=dzby[0:95, :, 0:95],
                                    in1=dzby[0:95, :, 1:96], op=Alu.add)
            t = t95("t")
            nc.vector.tensor_tensor(out=t[0:95], in0=u[0:95], in1=v[0:95],
                                    op=Alu.subtract)
            jx = jp.tile(sh95, F16, tag="jx16", bufs=1, name="jx")
            nc.vector.tensor_scalar(out=jx[0:95], in0=t[0:95], scalar1=0.5,
                                    scalar2=None, op0=Alu.mult)
            nc.sync.dma_start(out=scr['jx'][:], in_=jx[0:95])

    with tc.tile_pool(name="p1b", bufs=1) as pool:
        keep = {}
        for (pref, J) in [("0", J0), ("1", J1)]:
            with tc.tile_pool(name=f"p1b_in{pref}", bufs=1) as ip:
                tj = {}
                for f in ['bxt', 'byt', 'bxp', 'byp']:
                    t = ip.tile(shape, F16, tag=f"J_{f}", bufs=1, name=f"tJ{pref}_{f}")
                    nc.sync.dma_start(out=t[0:96], in_=J[f][:])
                    tj[f] = t
                mk = _make_mask(nc, tc, ip, f"maskJ{pref}", shape, 96,
                                tj['bxp'][0:96], tj['byp'][0:96],
                                tj['bxt'][0:96], tj['byt'][0:96], dt=F16)
                bxm = pool.tile(shape, F16, tag=f"bxm{pref}", bufs=1,
                                name=f"bxm{pref}")
                nc.vector.tensor_tensor(out=bxm[0:96], in0=tj['bxt'][0:96],
                                        in1=mk[0:96], op=Alu.mult)
                bym = pool.tile(shape, F16, tag=f"bym{pref}", bufs=1,
                                name=f"bym{pref}")
                nc.vector.tensor_tensor(out=bym[0:96], in0=tj['byt'][0:96],
                                        in1=mk[0:96], op=Alu.mult)
                keep[f"bxm{pref}"] = bxm
                keep[f"bym{pref}"] = bym

        bzp0 = pool.tile(shape, F16, tag="bzp0", bufs=1, name="bzp0")
        nc.sync.dma_start(out=bzp0[0:96], in_=J0['bzp'][:])
        bzp1 = pool.tile(shape, F16, tag="bzp1", bufs=1, name="bzp1")
        nc.sync.dma_start(out=bzp1[0:96], in_=J1['bzp'][:])

        with tc.tile_pool(name="p1b_j", bufs=1) as jp:
            def tmp(name):
                return jp.tile(shape, F16, tag="tmp", bufs=7, name=name)

            def t95(name):
                return jp.tile(sh95, F16, tag="t95", bufs=6, name=name)

            bxm0, bxm1 = keep["bxm0"], keep["bxm1"]
            bym0, bym1 = keep["bym0"], keep["bym1"]
            # jy = 0.5*[(dzBx0 + dzBx1) - (dxz + dxz_s)], valid rows 0..94
            bxm0S = tmp("bxm0S")
            nc.sync.dma_start(out=bxm0S[0:95], in_=bxm0[1:96])
            bxm1S = tmp("bxm1S")
            nc.sync.dma_start(out=bxm1S[0:95], in_=bxm1[1:96])
            dzbx0 = tmp("dzbx0")
            nc.vector.tensor_tensor(out=dzbx0[0:95], in0=bxm0[0:95], in1=bxm0S[0:95],
                                    op=Alu.subtract)
            dzbx1 = tmp("dzbx1")
            nc.vector.tensor_tensor(out=dzbx1[0:95], in0=bxm1[0:95], in1=bxm1S[0:95],
                                    op=Alu.subtract)
            a = tmp("a")
            nc.vector.tensor_tensor(out=a[0:95], in0=dzbx0[0:95], in1=dzbx1[0:95],
                                    op=Alu.add)
            dxz = tmp("dxz")
            nc.vector.tensor_tensor(out=dxz[0:96], in0=bzp0[0:96], in1=bzp1[0:96],
                                    op=Alu.subtract)
            dxzS = tmp("dxzS")
            nc.sync.dma_start(out=dxzS[0:95], in_=dxz[1:96])
            b = tmp("b")
            nc.vector.tensor_tensor(out=b[0:95], in0=dxz[0:95], in1=dxzS[0:95],
                                    op=Alu.add)
            t2 = tmp("t2")
            nc.vector.tensor_tensor(out=t2[0:95], in0=a[0:95], in1=b[0:95],
                                    op=Alu.subtract)
            jy = jp.tile(shape, F16, tag="jy16", bufs=1, name="jy")
            nc.vector.tensor_scalar(out=jy[0:95], in0=t2[0:95], scalar1=0.5,
                                    scalar2=None, op0=Alu.mult)
            nc.sync.dma_start(out=scr['jy'][:], in_=jy[0:95])

            # jz = 0.5*[(dxBy[h] + dxBy[h+1]) - (dyBx0 + dyBx1)], rows 0..95
            dxby = tmp("dxby")
            nc.vector.tensor_tensor(out=dxby[0:96], in0=bym0[0:96], in1=bym1[0:96],
                                    op=Alu.subtract)
            aa = t95("aa")
            nc.vector.tensor_tensor(out=aa[0:96], in0=dxby[0:96, :, 0:95],
                                    in1=dxby[0:96, :, 1:96], op=Alu.add)
            dybx0 = t95("dybx0")
            nc.vector.tensor_tensor(out=dybx0[0:96], in0=bxm0[0:96, :, 0:95],
                                    in1=bxm0[0:96, :, 1:96], op=Alu.subtract)
            dybx1 = t95("dybx1")
            nc.vector.tensor_tensor(out=dybx1[0:96], in0=bxm1[0:96, :, 0:95],
                                    in1=bxm1[0:96, :, 1:96], op=Alu.subtract)
            bb = t95("bb")
            nc.vector.tensor_tensor(out=bb[0:96], in0=dybx0[0:96], in1=dybx1[0:96],
                                    op=Alu.add)
            tt = t95("tt")
            nc.vector.tensor_tensor(out=tt[0:96], in0=aa[0:96], in1=bb[0:96],
                                    op=Alu.subtract)
            jz = jp.tile(sh95, F16, tag="jz16", bufs=1, name="jz")
            nc.vector.tensor_scalar(out=jz[0:96], in0=tt[0:96], scalar1=0.5,
                                    scalar2=None, op0=Alu.mult)
            nc.sync.dma_start(out=scr['jz'][:], in_=jz[0:96])


def _emit_pass1_flux(nc, tc, FX, acc, maskp):
    """cal_div_c_old for both variants; accumulate Sf, Sf2 into acc slots.
    All flux math on partition rows [0:95) (corner W extent)."""
    nc.tc = tc
    shape = [128, 13, 96]
    NR = 95

    with tc.tile_pool(name="flux", bufs=1) as pool:
        T, TS = {}, {}
        for f in ['bxp', 'byp', 'bxt', 'byt', 'bzt', 'z']:
            t = pool.tile(shape, F32, tag=f"T_{f}", bufs=1, name=f"T_{f}")
            nc.sync.dma_start(out=t[0:96], in_=FX[f][:])
            T[f] = t
            s = pool.tile(shape, F32, tag=f"S_{f}", bufs=1, name=f"S_{f}")
            nc.sync.dma_start(out=s[0:95], in_=FX[f][1:96])
            TS[f] = s

        maskT = _make_mask(nc, tc, pool, "maskT", shape, NR, T['bxp'][0:NR],
                           T['byp'][0:NR], T['bxt'][0:NR], T['byt'][0:NR])
        maskS = _make_mask(nc, tc, pool, "maskS", shape, NR, TS['bxp'][0:NR],
                           TS['byp'][0:NR], TS['bxt'][0:NR], TS['byt'][0:NR])
        bxmT = pool.tile(shape, F32, tag="bxmT", bufs=1, name="bxmT")
        nc.vector.tensor_tensor(out=bxmT[0:NR], in0=T['bxt'][0:NR],
                                in1=maskT[0:NR], op=Alu.mult)
        bymT = pool.tile(shape, F32, tag="bymT", bufs=1, name="bymT")
        nc.vector.tensor_tensor(out=bymT[0:NR], in0=T['byt'][0:NR],
                                in1=maskT[0:NR], op=Alu.mult)
        bxmS = pool.tile(shape, F32, tag="bxmS", bufs=1, name="bxmS")
        nc.vector.tensor_tensor(out=bxmS[0:NR], in0=TS['bxt'][0:NR],
                                in1=maskS[0:NR], op=Alu.mult)
        bymS = pool.tile(shape, F32, tag="bymS", bufs=1, name="bymS")
        nc.vector.tensor_tensor(out=bymS[0:NR], in0=TS['byt'][0:NR],
                                in1=maskS[0:NR], op=Alu.mult)
        Tm = {'bx': bxmT, 'by': bymT}
        TSm = {'bx': bxmS, 'by': bymS}

        for (h0, hcf) in chunks_for(95, HCF):
            _emit_flux_chunk(nc, tc, T, TS, Tm, TSm, acc, maskp, h0, hcf, NR)


def _emit_flux_chunk(nc, tc, T, TS, Tm, TSm, acc, maskp, h0, hcf, NR):
    cs = [128, 12, hcf]

    def C(fld, i, j, l):
        base = TS[fld] if l == 1 else T[fld]
        return base[0:NR, i:i + 12, h0 + j:h0 + j + hcf]

    def Cv(variant, xy, i, j, l):
        if variant == 'p':
            return C('bxp' if xy == 'bx' else 'byp', i, j, l)
        base = TSm[xy] if l == 1 else Tm[xy]
        return base[0:NR, i:i + 12, h0 + j:h0 + j + hcf]

    with tc.tile_pool(name=f"fxc_{h0}", bufs=1) as pool:
        def mk(tag, bufs, name, dt=F32):
            return pool.tile(cs, dt, tag=tag, bufs=bufs, name=name)

        def tt(op, a, b, tag, bufs):
            o = mk(tag, bufs, f"{tag}_o")
            nc.vector.tensor_tensor(out=o[0:NR], in0=a, in1=b, op=op)
            return o[0:NR]

        def ts(op, a, s1, s2=None, op2=None, tag="v", bufs=26):
            o = mk(tag, bufs, f"{tag}_s")
            nc.vector.tensor_scalar(out=o[0:NR], in0=a, scalar1=s1, scalar2=s2,
                                    op0=op, op1=op2 if op2 else Alu.bypass)
            return o[0:NR]

        # shared z pieces
        za = {}
        for (i, j) in [(0, 0), (0, 1), (1, 0), (1, 1)]:
            d = tt(Alu.subtract, C('z', i, j, 1), C('z', i, j, 0), "za", 10)
            o = mk("za", 10, "za_abs")
            nc.scalar.activation(out=o[0:NR], in_=d, func=ActF.Abs)
            za[(i, j)] = o[0:NR]
        P1 = tt(Alu.add, za[(1, 0)], za[(1, 1)], "za", 10)
        P0 = tt(Alu.add, za[(0, 0)], za[(0, 1)], "za", 10)
        PH1 = tt(Alu.add, za[(0, 1)], za[(1, 1)], "za", 10)
        PH0 = tt(Alu.add, za[(0, 0)], za[(1, 0)], "za", 10)
        zd01 = tt(Alu.subtract, C('z', 0, 0, 1), C('z', 1, 0, 1), "zt", 9)
        zd11 = tt(Alu.subtract, C('z', 0, 1, 1), C('z', 1, 1, 1), "zt", 9)
        zh11 = tt(Alu.subtract, C('z', 1, 0, 1), C('z', 1, 1, 1), "zt", 9)
        zh01 = tt(Alu.subtract, C('z', 0, 0, 1), C('z', 0, 1, 1), "zt", 9)
        zd00 = tt(Alu.subtract, C('z', 0, 0, 0), C('z', 1, 0, 0), "zt", 9)
        zdd10 = tt(Alu.subtract, C('z', 0, 1, 0), C('z', 1, 1, 0), "zt", 9)
        zhh10 = tt(Alu.subtract, C('z', 1, 0, 0), C('z', 1, 1, 0), "zt", 9)
        zh00 = tt(Alu.subtract, C('z', 0, 0, 0), C('z', 0, 1, 0), "zt", 9)

        def sum_corners(get, corners, tag, bufs):
            o = tt(Alu.add, get(*corners[0]), get(*corners[1]), tag, bufs)
            for c in corners[2:]:
                o = tt(Alu.add, o, get(*c), tag, bufs)
            return o

        def Cz(i, j, l):
            return C('bzt', i, j, l)

        t1a = sum_corners(Cz, [(0, 0, 1), (1, 0, 1), (1, 1, 1)], "bz", 11)
        t1b = sum_corners(Cz, [(0, 0, 1), (1, 1, 1), (0, 1, 1)], "bz", 11)
        bzs1 = tt(Alu.add, t1a, t1b, "bz", 11)
        t0a = sum_corners(Cz, [(0, 0, 0), (1, 0, 0), (1, 1, 0)], "bz", 11)
        t0b = sum_corners(Cz, [(0, 0, 0), (1, 1, 0), (0, 1, 0)], "bz", 11)
        bzs0 = tt(Alu.add, t0a, t0b, "bz", 11)
        bzdiff = tt(Alu.subtract, bzs1, bzs0, "bz", 11)
        bz8 = sum_corners(Cz, [(i, j, l) for i in (0, 1) for j in (0, 1)
                               for l in (0, 1)], "bz", 11)
        bz8s = ts(Alu.mult, bz8, 0.125, tag="bz", bufs=11)
        bz8sq = tt(Alu.mult, bz8s, bz8s, "bz", 11)

        for variant in ['p', 't']:
            def Cx(i, j, l, _v=variant):
                return Cv(_v, 'bx', i, j, l)

            def Cy(i, j, l, _v=variant):
                return Cv(_v, 'by', i, j, l)

            V = ("v", 26)
            bxs1 = sum_corners(Cx, [(1, 0, 0), (1, 1, 0), (1, 0, 1), (1, 1, 1)], *V)
            bxs0 = sum_corners(Cx, [(0, 0, 0), (0, 1, 0), (0, 0, 1), (0, 1, 1)], *V)
            bysj1 = sum_corners(Cy, [(0, 1, 0), (1, 1, 0), (0, 1, 1), (1, 1, 1)], *V)
            bysj0 = sum_corners(Cy, [(0, 0, 0), (1, 0, 0), (0, 0, 1), (1, 0, 1)], *V)
            x1a = sum_corners(Cx, [(0, 0, 1), (1, 0, 1), (1, 1, 1)], *V)
            x1b = sum_corners(Cx, [(0, 0, 1), (0, 1, 1), (1, 1, 1)], *V)
            x0a = sum_corners(Cx, [(0, 0, 0), (1, 0, 0), (1, 1, 0)], *V)
            x0b = sum_corners(Cx, [(0, 0, 0), (0, 1, 0), (1, 1, 0)], *V)
            y1a = sum_corners(Cy, [(0, 0, 1), (1, 0, 1), (1, 1, 1)], *V)
            y1b = sum_corners(Cy, [(0, 0, 1), (0, 1, 1), (1, 1, 1)], *V)
            y0a = sum_corners(Cy, [(0, 0, 0), (1, 0, 0), (1, 1, 0)], *V)
            y0b = sum_corners(Cy, [(0, 0, 0), (0, 1, 0), (1, 1, 0)], *V)

            g1 = tt(Alu.mult, bxs1, P1, *V)
            g2 = tt(Alu.mult, bxs0, P0, *V)
            gA = tt(Alu.subtract, g1, g2, *V)
            g3 = tt(Alu.mult, bysj1, PH1, *V)
            g4 = tt(Alu.mult, bysj0, PH0, *V)
            gB = tt(Alu.add, gA, g3, *V)
            gC = tt(Alu.subtract, gB, g4, *V)

            h1 = tt(Alu.mult, x1a, zd01, *V)
            h2 = tt(Alu.mult, x1b, zd11, *V)
            hA = tt(Alu.add, h1, h2, *V)
            h3 = tt(Alu.mult, y1a, zh11, *V)
            h4 = tt(Alu.mult, y1b, zh01, *V)
            hB = tt(Alu.add, h3, h4, *V)
            hAB = tt(Alu.add, hA, hB, *V)
            h5 = tt(Alu.mult, x0a, zd00, *V)
            h6 = tt(Alu.mult, x0b, zdd10, *V)
            hC = tt(Alu.add, h5, h6, *V)
            h7 = tt(Alu.mult, y0a, zhh10, *V)
            h8 = tt(Alu.mult, y0b, zh00, *V)
            hD = tt(Alu.add, h7, h8, *V)
            hCD = tt(Alu.add, hC, hD, *V)
            hdiff = tt(Alu.subtract, hAB, hCD, *V)
            hfull = tt(Alu.add, hdiff, bzdiff, *V)

            gs = ts(Alu.mult, gC, 0.125, tag="v", bufs=26)
            hs = ts(Alu.mult, hfull, 1.0 / 6.0, tag="v", bufs=26)
            flux = tt(Alu.add, gs, hs, *V)

            res2 = mk("vf32", 8, "res2", dt=F32)
            nc.vector.tensor_tensor(out=res2[0:NR], in0=flux, in1=flux,
                                    op=Alu.mult)
            res4 = mk("vf32", 8, "res4", dt=F32)
            nc.vector.tensor_tensor(out=res4[0:NR], in0=res2[0:NR],
                                    in1=res2[0:NR], op=Alu.mult)
            res4 = res4[0:NR]
            bx8 = tt(Alu.add, bxs1, bxs0, *V)
            bx8s = ts(Alu.mult, bx8, 0.125, tag="v", bufs=26)
            bx8sq = tt(Alu.mult, bx8s, bx8s, *V)
            by8 = tt(Alu.add, bysj1, bysj0, *V)
            by8s = ts(Alu.mult, by8, 0.125, tag="v", bufs=26)
            by8sq = tt(Alu.mult, by8s, by8s, *V)
            ab1 = tt(Alu.add, bx8sq, by8sq, *V)
            ab2 = tt(Alu.add, ab1, bz8sq, *V)
            aveb = mk("vf32", 8, "aveb", dt=F32)
            nc.vector.tensor_scalar(out=aveb[0:NR], in0=ab2, scalar1=1e-8,
                                    scalar2=None, op0=Alu.add)
            # divide is unsupported: reciprocal (ACT) + one Newton step
            rcp = mk("vf32", 8, "rcp", dt=F32)
            nc.vector.reciprocal(out=rcp[0:NR], in_=aveb[0:NR])
            ar = mk("vf32", 8, "ar", dt=F32)
            nc.vector.tensor_tensor(out=ar[0:NR], in0=aveb[0:NR], in1=rcp[0:NR],
                                    op=Alu.mult)
            two_m = mk("vf32", 8, "two_m", dt=F32)
            nc.vector.tensor_scalar(out=two_m[0:NR], in0=ar[0:NR], scalar1=-1.0,
                                    scalar2=2.0, op0=Alu.mult, op1=Alu.add)
            rcp2 = mk("vf32", 8, "rcp2", dt=F32)
            nc.vector.tensor_tensor(out=rcp2[0:NR], in0=rcp[0:NR],
                                    in1=two_m[0:NR], op=Alu.mult)
            flx1 = mk("vf32", 8, "flx1", dt=F32)
            nc.vector.tensor_tensor(out=flx1[0:NR], in0=res4,
                                    in1=rcp2[0:NR], op=Alu.mult)
            flx1 = flx1[0:NR]

            _acc_masked_sums(nc, pool, acc, maskp, flx1, cs, NR,
                             SLOT[f'f_{variant}'], SLOT[f'f2_{variant}'],
                             nplanes=12, mask_last=True)


def _acc_masked_sums(nc, pool, acc, maskp, fld, fshape, NR, slot1, slot2, nplanes,
                     mask_last):
    """acc[slot1] += sum(fld), acc[slot2] += sum(fld^2); optional mask on the
    last plane. fld: AP [NR, nplanes, X]."""
    sq = pool.tile(fshape, F32, tag="sq", bufs=2, name="sq")
    nc.scalar.activation(out=sq[0:NR], in_=fld, func=ActF.Square)

    def r(name):
        return pool.tile([128, 1], F32, tag="r", bufs=8, name=name)

    for (slot, fsrc) in [(slot1, fld), (slot2, sq[0:NR])]:
        ra = r("ra")
        nc.vector.tensor_reduce(out=ra[0:NR], in_=fsrc[:, 0:nplanes - 1, :],
                                axis=mybir.AxisListType.XY, op=Alu.add)
        rb = r("rb")
        nc.vector.tensor_reduce(out=rb[0:NR], in_=fsrc[:, nplanes - 1:nplanes, :],
                                axis=mybir.AxisListType.XY, op=Alu.add)
        if mask_last:
            rbm = r("rbm")
            nc.vector.tensor_tensor(out=rbm[0:NR], in0=rb[0:NR], in1=maskp[0:NR],
                                    op=Alu.mult)
            rb = rbm
        rs = r("rs")
        nc.vector.tensor_tensor(out=rs[0:NR], in0=ra[0:NR], in1=rb[0:NR], op=Alu.add)
        nc.vector.tensor_tensor(out=acc[0:NR, slot:slot + 1],
                                in0=acc[0:NR, slot:slot + 1],
                                in1=rs[0:NR], op=Alu.add)


_ENG_CTR = [0]


def _med_eng(nc):
    """Weighted engine rotation for the median CE networks."""
    _ENG_CTR[0] += 1
    if GP_MOD and _ENG_CTR[0] % GP_MOD == 0:
        return nc.gpsimd
    return nc.vector


def _emit_pass2_medians(nc, tc, AH, scr, acc):
    nc.tc = tc
    src_map = {
        'bxp': (AH['bxp'], 96, 96),
        'byp': (AH['byp'], 96, 96),
        'bxm': (scr['bxm'], 96, 96),
        'bym': (scr['bym'], 96, 96),
        'jx': (scr['jx'], 95, 95),
        'jy': (scr['jy'], 95, 96),
        'jz': (scr['jz'], 96, 95),
    }
    with tc.tile_pool(name="medglobal", bufs=1) as pool:
        for fname in FILTER_ORDER:
            dram, Wext, Hext = src_map[fname]
            mask_last = fname in ('jy', 'jz')
            _emit_one_median(nc, tc, pool, fname, dram, Wext, Hext, acc,
                             mask_last)


def _emit_one_median(nc, tc, pool, fname, dram, Wext, Hext, acc, mask_last):
    He = Hext + 4
    NR = Wext
    # s25f rotates per-field within one shared 25-buffer set; X tiles are
    # double-buffered so the next field's loads overlap this field's compute
    s25f = [pool.tile([128, DIN, 96], F16, tag="s25f", bufs=25,
                      name=f"s25f_{i}") for i in range(25)]
    X = []
    for k in range(5):
        xk = pool.tile([128, DIN, 100], F16, tag=f"X{k}",
                       bufs=2 if k == 2 else 1, name=f"X{fname}{k}")
        p_lo = max(0, 2 - k)
        p_hi = min(Wext, Wext + 2 - k)
        nc.sync.dma_start(out=xk[p_lo:p_hi, :, 2:2 + Hext],
                          in_=dram[p_lo + k - 2:p_hi + k - 2, :, :])
        for p in list(range(0, p_lo)) + list(range(p_hi, Wext)):
            w = refl(p + k - 2, Wext)
            nc.sync.dma_start(out=xk[p:p + 1, :, 2:2 + Hext],
                              in_=dram[w:w + 1, :, :])
        for (dst, srcc) in [(0, 4), (1, 3), (2 + Hext, Hext),
                            (3 + Hext, Hext - 1)]:
            nc.sync.dma_start(out=xk[0:Wext, :, dst:dst + 1],
                              in_=xk[0:Wext, :, srcc:srcc + 1])
        X.append(xk)

    for (h0, hc) in chunks_for(Hext, HC):
        _emit_phase2_chunk(nc, tc, X, h0, hc, NR, s25f)

    for (h0, hc) in chunks_for(Hext, HC3):
        _emit_phase3_chunk(nc, tc, s25f, X[2], h0, hc, NR, acc, fname,
                           mask_last)


def _emit_phase2_chunk(nc, tc, X, h0, hc, NR, s25f):
    hc4 = hc + 4
    with tc.tile_pool(name=f"mp2_{h0}", bufs=1) as p2:
        wires = {i: X[i][0:NR, :, h0:h0 + hc4] for i in range(5)}
        emit_ces_nr(nc, p2, "s", 9, [128, DIN, hc4], NR, wires,
                    _prune(SORT5, set(range(5))))
        s = [wires[i] for i in range(5)]

        ces, order = merge_net(5, 5)
        w = {}
        for i in range(5):
            w[i] = s[i][:, :, 0:hc + 3]
            w[5 + i] = s[i][:, :, 1:hc4]
        emit_ces_nr(nc, p2, "p10", 12, [128, DIN, hc + 3], NR, w, ces)
        P10 = [w[o] for o in order]

        ces, order = merge_net(10, 5)
        w = {}
        for i in range(10):
            w[i] = P10[i][:, :, 0:hc + 2]
        for i in range(5):
            w[10 + i] = s[i][:, :, 2:hc4]
        emit_ces_nr(nc, p2, "t15", 17, [128, DIN, hc + 2], NR, w, ces)
        T15 = [w[o] for o in order]

        ces, order = merge_net(15, 10)
        w = {}
        for i in range(15):
            w[i] = T15[i][:, :, 2:hc + 2]
        for i in range(10):
            w[15 + i] = P10[i][:, :, 0:hc]
        # final writers land directly in the persistent full-H S25 slices
        final_dst = {order[r]: s25f[r][0:NR, :, h0:h0 + hc] for r in range(25)}
        emit_ces_nr(nc, p2, "s25", 26, [128, DIN, hc], NR, w, ces,
                    final_dst=final_dst)


def _emit_phase3_chunk(nc, tc, s25f, x2, h0, hc, NR, acc, fname, mask_last):
    S25 = [s25f[i][0:NR, :, h0:h0 + hc] for i in range(25)]
    X2 = x2

    with tc.tile_pool(name=f"mp3_{h0}", bufs=1) as p3:
        if True:
            ces, order = merge_net(25, 25)
            w = {}
            for i in range(25):
                w[i] = S25[i][:, 1:14:2, :]
                w[25 + i] = S25[i][:, 2:15:2, :]
            emit_ces_nr(nc, p3, "m2", 52, [128, 7, hc], NR, w, ces)
            M2 = [w[o] for o in order]

            ces, order = merge_net(50, 50, needed_ranks=range(37, 63))
            w = {}
            for i in range(50):
                w[i] = M2[i][:, 0:6, :]
                w[50 + i] = M2[i][:, 1:7, :]
            emit_ces_nr(nc, p3, "q", 31, [128, 6, hc], NR, w, ces)
            Q = {r: w[order[r]] for r in range(37, 63)}

            meds = []
            for par, sel in [(0, slice(0, 11, 2)), (1, slice(5, 16, 2))]:
                Sside = [S25[i][:, sel, :] for i in range(25)]
                accw = Q[62]
                for j in range(1, 26):
                    t = p3.tile([128, 6, hc], F16, tag="sel", bufs=6, name="sel_t")
                    _med_eng(nc).tensor_tensor(out=t[0:NR], in0=Q[62 - j],
                                               in1=Sside[j - 1], op=Alu.max)
                    t2 = p3.tile([128, 6, hc], F16, tag="sel", bufs=6, name="sel_m")
                    _med_eng(nc).tensor_tensor(out=t2[0:NR], in0=accw, in1=t[0:NR],
                                               op=Alu.min)
                    accw = t2[0:NR]
                meds.append(accw)

            for par, med in enumerate(meds):
                xs = X2[0:NR, 2 + par:14 + par:2, h0 + 2:h0 + 2 + hc]
                d = p3.tile([128, 6, hc], F16, tag="sel", bufs=6, name="d")
                _med_eng(nc).tensor_tensor(out=d[0:NR], in0=med, in1=xs,
                                           op=Alu.subtract)
                d2 = p3.tile([128, 6, hc], F16, tag="sel", bufs=6, name="d2")
                if par == 1 and mask_last:
                    cm = MED_SLOT[(fname, h0, par, 'm')]
                    nc.scalar.activation(out=d2[0:NR, 0:5, :], in_=d[0:NR, 0:5, :],
                                         func=ActF.Square,
                                         accum_out=acc[0:NR, cm:cm + 1])
                    cl = MED_SLOT[(fname, h0, par, 'l')]
                    nc.scalar.activation(out=d2[0:NR, 5:6, :], in_=d[0:NR, 5:6, :],
                                         func=ActF.Square,
                                         accum_out=acc[0:NR, cl:cl + 1])
                else:
                    cm = MED_SLOT[(fname, h0, par, 'm')]
                    nc.scalar.activation(out=d2[0:NR], in_=d[0:NR],
                                         func=ActF.Square,
                                         accum_out=acc[0:NR, cm:cm + 1])


def emit_ces_nr(nc, pool, tag, bufs, shape, NR, wires, ces, final_dst=None):
    """SSA compare-exchange emission on partition rows [0:NR). Wires listed in
    final_dst have their LAST write redirected to the given AP (which must
    already be partition-sliced to [0:NR))."""
    last_write = {}
    if final_dst:
        for idx, (u, v, nm, nM) in enumerate(ces):
            if nm:
                last_write[u] = idx
            if nM:
                last_write[v] = idx
    for idx, (u, v, nm, nM) in enumerate(ces):
        a, b = wires[u], wires[v]
        dmn = dmx = None
        if final_dst:
            if nm and u in final_dst and last_write.get(u) == idx:
                dmn = final_dst[u]
            if nM and v in final_dst and last_write.get(v) == idx:
                dmx = final_dst[v]
        if nm:
            if dmn is None:
                tmn = pool.tile(shape, F16, tag=tag, bufs=bufs, name=f"{tag}_mn")
                dmn = tmn[0:NR]
            _med_eng(nc).tensor_tensor(out=dmn, in0=a, in1=b, op=Alu.min)
        if nM:
            if dmx is None:
                tmx = pool.tile(shape, F16, tag=tag, bufs=bufs, name=f"{tag}_mx")
                dmx = tmx[0:NR]
            _med_eng(nc).tensor_tensor(out=dmx, in0=a, in1=b, op=Alu.max)
        if nm:
            wires[u] = dmn
        if nM:
            wires[v] = dmx


# ---------------------------------------------------------------------------
# host side
# ---------------------------------------------------------------------------

def _arrange(f, idx):
    """f: [D, H, W] -> [W, len(idx), H] contiguous."""
    return np.ascontiguousarray(np.asarray(f)[np.asarray(idx)].transpose(2, 0, 1))


def make_in_maps(pred_b, pred_z, targets):
    pb = np.asarray(pred_b, dtype=np.float32)[0]
    pz = np.asarray(pred_z, dtype=np.float32)[0, 0]
    tg = np.asarray(targets, dtype=np.float32)[0]
    fields = {
        'bxp': pb[0], 'byp': pb[1], 'bzp': pb[2],
        'bxt': tg[0], 'byt': tg[1], 'bzt': tg[2],
        'z': pz,
    }
    in_maps = []
    for c in range(NCORES):
        m = {}
        a_idx = [refl(12 * c - 2 + s, 96) for s in range(DIN)]
        jg = [refl(12 * c - 2 + s, 95) for s in range(DIN)]
        j1_idx = [g + 1 for g in jg]
        fx_idx = [min(12 * c + s, 95) for s in range(13)]
        for f in ['bxt', 'byt', 'bxp', 'byp', 'bzp']:
            m[f"A_{f}"] = _arrange(fields[f], a_idx).astype(np.float16)
            m[f"J0_{f}"] = _arrange(fields[f], jg).astype(np.float16)
            m[f"J1_{f}"] = _arrange(fields[f], j1_idx).astype(np.float16)
        for f in ['bxp', 'byp', 'bxt', 'byt', 'bzt', 'z']:
            m[f"Fx_{f}"] = _arrange(fields[f], fx_idx)
        m["Ah_bxp"] = m["A_bxp"]
        m["Ah_byp"] = m["A_byp"]
        mp = np.zeros((128, 1), dtype=np.float32)
        mp[:] = 0.0 if c == NCORES - 1 else 1.0
        m["maskp"] = mp
        in_maps.append(m)
    return in_maps


def combine(outs):
    """outs: list of 8 arrays [128, NSLOT] -> 6-scalar loss tuple."""
    def tot(slot, we):
        return float(sum(np.asarray(o)[:we, slot].astype(np.float64).sum()
                         for o in outs))

    def med_tot(fname, we):
        s = 0.0
        for (fn, h0, par, blk), col in MED_SLOT.items():
            if fn != fname:
                continue
            for ci, o in enumerate(outs):
                if blk == 'l' and ci == NCORES - 1:
                    continue
                s += float(np.asarray(o)[:we, col].astype(np.float64).sum())
        return s

    N95 = 95.0 ** 3
    s_fp = tot(SLOT['f_p'], 95)
    s_f2p = tot(SLOT['f2_p'], 95)
    s_ft = tot(SLOT['f_t'], 95)
    s_f2t = tot(SLOT['f2_t'], 95)
    loss_div_p = s_fp / N95
    std_p = s_f2p / N95 - loss_div_p ** 2
    loss_div_t = s_ft / N95
    std_t = s_f2t / N95 - loss_div_t ** 2
    loss_j = (med_tot('jx', 95) / (96 * 95 * 95)
              + med_tot('jy', 95) / (95 * 96 * 95)
              + med_tot('jz', 96) / (95 * 95 * 96))
    N96 = 96.0 ** 3
    loss_b = (med_tot('bxm', 96) + med_tot('bym', 96)
              + med_tot('bxp', 96) + med_tot('byp', 96)) / N96
    return (np.float32(loss_div_p), np.float32(std_p), np.float32(loss_div_t),
            np.float32(std_t), np.float32(loss_j), np.float32(loss_b))


_NC_CACHE = None


def get_program():
    """Program for hardware execution (multi-wait legalized)."""
    global _NC_CACHE
    if _NC_CACHE is None:
        nc = build_program()
        _legalize_multiwaits(nc)
        _NC_CACHE = nc
    return _NC_CACHE


def kernel(pred_b, pred_z, targets, iepoch=None, epoch_max=None):
    nc = get_program()
    in_maps = make_in_maps(pred_b, pred_z, targets)
    res = run_bass_kernel_spmd(nc, in_maps, list(range(NCORES)))
    outs = [res.results[i]["out"] for i in range(NCORES)]
    return combine(outs)

